# revision 22
# baseline (speedup 1.0000x reference)
"""v6: host-transposed x (no PE transposes), f32r direct-DMA projections,
bf16 attention + output path, paired MM2 row-groups with merged 2-bank
PSUM exp, LDW-reuse loop orders, f32 normalize chain, split DMA queues."""
import numpy as np
import concourse.bass as bass
import concourse.mybir as mybir
import concourse.tile as tile

dt = mybir.dt
F32 = dt.float32
F32R = dt.float32r
BF16 = dt.bfloat16
AF = mybir.ActivationFunctionType
ALU = mybir.AluOpType

B = 4            # batches per core
T = 577
D = 768
H = 12
HD = 64
EQK = 1536
SCALE = HD ** -0.5
NTOK = B * T

TT = [(i * 128, min(128, T - i * 128)) for i in range((T + 127) // 128)]
TP = 578         # padded token count (free dim)
ICH = [(0, 320), (320, 258)]       # f32r moving chunks (>=256 each)
ICHV = [(0, 320), (320, 257)]      # valid (non-pad) widths
BCH = [(0, 512), (512, 66)]        # bf16 moving chunks (bank-aligned)
ECH = [(0, 384), (384, 384)]
DT = 6


def build(nbatch=B, fastrecip=True):
    nc = bass.Bass()
    xT_d = nc.dram_tensor("xT", [nbatch * D, T], F32R, kind="ExternalInput")
    qkwT_d = nc.dram_tensor("qkwT", [D, EQK], F32R, kind="ExternalInput")
    vwT_d = nc.dram_tensor("vwT", [D, D], F32R, kind="ExternalInput")
    pwT_d = nc.dram_tensor("pwT", [D + 1, D], F32, kind="ExternalInput")
    qkb_d = nc.dram_tensor("qkb", [128, H], F32, kind="ExternalInput")
    y_d = nc.dram_tensor("y", [NTOK, D], F32, kind="ExternalOutput")

    from contextlib import ExitStack
    with tile.TileContext(nc) as tc, ExitStack() as ctx:
        wpool = ctx.enter_context(tc.tile_pool(name="wpool", bufs=1))
        stg = ctx.enter_context(tc.tile_pool(name="stg", bufs=2))

        # projection weights: f32r, straight from DRAM (same bytes as f32)
        qkwT = []
        for dti in range(DT):
            w = wpool.tile([128, EQK], F32R, tag=f"qkwT{dti}", name=f"qkwT{dti}")
            nc.sync.dma_start(w[:], qkwT_d[dti * 128:(dti + 1) * 128, :])
            qkwT.append(w)
        vwT = []
        for dti in range(DT):
            w = wpool.tile([128, D], F32R, tag=f"vwT{dti}", name=f"vwT{dti}")
            nc.sync.dma_start(w[:], vwT_d[dti * 128:(dti + 1) * 128, :])
            vwT.append(w)
        # output projection: bf16 (matches bf16 oT stationary)
        pwT = []
        for dti in range(DT):
            w = wpool.tile([128, D], BF16, tag=f"pwT{dti}", name=f"pwT{dti}")
            s = stg.tile([128, D], F32, tag="wstage", name=f"sp{dti}")
            nc.sync.dma_start(s[:], pwT_d[dti * 128:(dti + 1) * 128, :])
            nc.vector.tensor_copy(w[:], s[:])
            pwT.append(w)
        # proj bias row, broadcast to all partitions via DRAM bounce (one-time)
        pb_bc = wpool.tile([128, D], F32, tag="pb_bc")
        s = stg.tile([128, D], F32, tag="wstage", name="spb")
        nc.sync.dma_start(s[0:1, :], pwT_d[D:D + 1, :])
        pb_row = wpool.tile([1, D], F32, tag="pb_row")
        nc.vector.tensor_copy(pb_row[:], s[0:1, :])

        qkb_sb = wpool.tile([128, H], F32, tag="qkb")
        nc.sync.dma_start(qkb_sb[:], qkb_d[:])

        ones_col_f = wpool.tile([128, 1], F32, tag="ones_col_f")
        nc.gpsimd.memset(ones_col_f[:], 1.0)
        ones_col = wpool.tile([128, 1], BF16, tag="ones_col")
        nc.vector.tensor_copy(ones_col[:], ones_col_f[:])

        xT_p = ctx.enter_context(tc.tile_pool(name="xT", bufs=2))
        qkT_p = ctx.enter_context(tc.tile_pool(name="qkT", bufs=2))
        v_p = ctx.enter_context(tc.tile_pool(name="v", bufs=2))
        es_p = ctx.enter_context(tc.tile_pool(name="es", bufs=1))
        oT_p = ctx.enter_context(tc.tile_pool(name="oT", bufs=2))
        nrm_p = ctx.enter_context(tc.tile_pool(name="nrm", bufs=2))
        bc_p = ctx.enter_context(tc.tile_pool(name="bc", bufs=2))
        yout = ctx.enter_context(tc.tile_pool(name="yout", bufs=2))
        drp = ctx.enter_context(tc.tile_pool(name="dr", bufs=2, space="DRAM"))
        pbd = drp.tile([1, D], F32, tag="pbd")
        nc.sync.dma_start(pbd[:], pb_row[:])
        nc.sync.dma_start(pb_bc[:], pbd[0:1, :].to_broadcast((128, D)))

        ps_s = ctx.enter_context(tc.tile_pool(name="ps_s", bufs=2, space="PSUM"))
        ps_mm = ctx.enter_context(tc.tile_pool(name="ps_mm", bufs=2, space="PSUM"))
        ps_o = ctx.enter_context(tc.tile_pool(name="ps_o", bufs=2, space="PSUM"))

        state = {}

        def stage1(b):
            # x^T tiles straight from DRAM (host pre-transposed)
            xT = [xT_p.tile([128, TP], F32R, tag=f"xT{dti}", name=f"xT{dti}_{b}")
                  for dti in range(DT)]
            for dti in range(DT):
                nc.gpsimd.memset(xT[dti][:, T:TP].bitcast(F32), 0.0)
                nc.sync.dma_start(xT[dti][:, 0:T],
                                  xT_d[b * D + dti * 128: b * D + (dti + 1) * 128, :])

            # MM1a: qkT[et] = (W_qk x^T)[et] + bias; q et 0-5, k et 6-11.
            # dti-outer chunk-inner: each LDWEIGHTS serves both chunk matmuls.
            qkT = [qkT_p.tile([128, TP], BF16, tag=f"qkT{et}", name=f"qkT{et}_{b}")
                   for et in range(12)]
            for et in range(12):
                pm = [ps_mm.tile([128, 512], F32, tag="ps_mm", name=f"pma_{b}_{et}_{ci}")
                      for ci in range(2)]
                for dti in range(DT):
                    for ci, (cs, cw) in enumerate(ICH):
                        nc.tensor.matmul(pm[ci][:, 0:cw],
                                         qkwT[dti][:, et * 128:(et + 1) * 128],
                                         xT[dti][:, cs:cs + cw],
                                         start=(dti == 0), stop=(dti == DT - 1))
                for ci, (cs, cw) in enumerate(ICH):
                    nc.vector.tensor_scalar_add(qkT[et][:, cs:cs + cw], pm[ci][:, 0:cw],
                                                qkb_sb[:, et:et + 1])

            # MM1b: v token-major bf16, per-head 64 cols + ones col (denominator)
            v_sb = [v_p.tile([128, H * (HD + 1)], BF16, tag=f"v{ti}", name=f"v{ti}_{b}")
                    for ti in range(len(TT))]
            for ti, (ts_, P) in enumerate(TT):
                vv = v_sb[ti].rearrange("p (h c) -> p h c", c=HD + 1)
                nc.vector.tensor_copy(vv[0:P, :, HD:HD + 1],
                                      ones_col[0:P, :].to_broadcast((P, H, 1)))
                pm = [ps_mm.tile([128, 512], F32, tag="ps_mm", name=f"pmb_{b}_{ti}_{ci}")
                      for ci in range(2)]
                for dti in range(DT):
                    for ci, (cs, cw) in enumerate(ECH):
                        nc.tensor.matmul(pm[ci][0:P, 0:cw],
                                         xT[dti][:, ts_:ts_ + P],
                                         vwT[dti][:, cs:cs + cw],
                                         start=(dti == 0), stop=(dti == DT - 1))
                for ci, (cs, cw) in enumerate(ECH):
                    s3 = pm[ci][0:P, 0:cw].rearrange("p (h c) -> p h c", c=HD)
                    nc.vector.tensor_copy(vv[0:P, ci * 6:(ci + 1) * 6, 0:HD], s3)

            state[b] = (qkT, v_sb)

        def attn(b):
            qkT, v_sb = state.pop(b)
            oT = [oT_p.tile([128, TP], BF16, tag=f"oT{dti}", name=f"oT{dti}_{b}")
                  for dti in range(DT)]
            rdr_den = drp.tile([H, TP], F32, tag="rdr_den", name=f"rdrden_{b}")
            for g in range(6):
                qt = qkT[g]
                kt = qkT[6 + g]
                # paired MM2: heads 2g (rows 0-63) / 2g+1 (rows 64-127) run in
                # separate PE row-groups concurrently; one exp per (head, jtile)
                es = {}
                for h01 in range(2):
                    h = 2 * g + h01
                    es[h01] = [es_p.tile([128, TP], BF16, tag=f"es{ji}_{h % 3}",
                                         name=f"es{ji}_{b}_{h}")
                               for ji in range(len(TT))]
                for ji, (js, JP) in enumerate(TT):
                    pss = [ps_s.tile([128, TP], F32, tag="ps_s",
                                     name=f"pss_{b}_{g}_{ji}_{h01}")
                           for h01 in range(2)]
                    for (cs, cw) in BCH:
                        for h01 in range(2):
                            par = h01 * 64
                            nc.tensor.matmul(pss[h01][0:JP, cs:cs + cw],
                                             kt[par:par + 64, js:js + JP],
                                             qt[par:par + 64, cs:cs + cw],
                                             start=True, stop=True)
                    for h01 in range(2):
                        nc.scalar.activation(es[h01][ji][0:JP, 0:TP],
                                             pss[h01][0:JP, 0:TP],
                                             AF.Exp, scale=SCALE)
                for h01 in range(2):
                    h = 2 * g + h01
                    par = h01 * 64
                    dh = nrm_p.tile([1, TP], F32, tag="dh", name=f"dh_{b}_{h}")
                    for (cs, cw), (_, cwv) in zip(ICH, ICHV):
                        po = ps_o.tile([128, 320], F32, tag="ps_o", name=f"po_{b}_{h}_{cs}")
                        for ji, (js, JP) in enumerate(TT):
                            nc.tensor.matmul(po[0:HD + 1, 0:cw],
                                             v_sb[ji][0:JP, h * (HD + 1):(h + 1) * (HD + 1)],
                                             es[h01][ji][0:JP, cs:cs + cw],
                                             start=(ji == 0), stop=(ji == len(TT) - 1))
                        nc.vector.tensor_copy(oT[g][par:par + 64, cs:cs + cwv],
                                              po[0:HD, 0:cwv])
                        nc.vector.tensor_copy(dh[0:1, cs:cs + cwv], po[HD:HD + 1, 0:cwv])
                    nc.sync.dma_start(rdr_den[h:h + 1, 0:T], dh[0:1, 0:T])

            # normalize: recip of denominators, broadcast per head-pair, scale oT
            den = nrm_p.tile([H, TP], F32, tag="den", name=f"den_{b}")
            nc.sync.dma_start(den[:, 0:T], rdr_den[:, 0:T])
            rec = nrm_p.tile([H, TP], F32, tag="rec", name=f"rec_{b}")
            if fastrecip:
                nc.vector.reciprocal_approx_fast(rec[:, 0:T], den[:, 0:T])
            else:
                nc.vector.reciprocal(rec[:, 0:T], den[:, 0:T])
            rdr = drp.tile([H, TP], F32, tag="rdr", name=f"rdr_{b}")
            nc.scalar.dma_start(rdr[:, 0:T], rec[:, 0:T])
            for g in range(6):
                bc = bc_p.tile([128, TP], F32, tag="bc", name=f"bc_{b}_{g}")
                nc.scalar.dma_start(bc[0:64, 0:T],
                                    rdr[2 * g:2 * g + 1, 0:T].to_broadcast((64, T)))
                nc.scalar.dma_start(bc[64:128, 0:T],
                                    rdr[2 * g + 1:2 * g + 2, 0:T].to_broadcast((64, T)))
                nc.vector.tensor_tensor(oT[g][:, 0:T], oT[g][:, 0:T],
                                        bc[:, 0:T], ALU.mult)

            # MM4: y = oT^T pwT + pb; dti-outer so LDW serves both chunks
            for ti, (ts_, P) in enumerate(TT):
                ys = yout.tile([128, D], F32, tag="y_sb", name=f"ys_{b}_{ti}")
                pm = [ps_mm.tile([128, 512], F32, tag="ps_mm", name=f"pmc_{b}_{ti}_{ci}")
                      for ci in range(2)]
                for dti in range(DT):
                    for ci, (cs, cw) in enumerate(ECH):
                        nc.tensor.matmul(pm[ci][0:P, 0:cw],
                                         oT[dti][:, ts_:ts_ + P],
                                         pwT[dti][:, cs:cs + cw],
                                         start=(dti == 0), stop=(dti == DT - 1))
                for ci, (cs, cw) in enumerate(ECH):
                    nc.vector.tensor_tensor(ys[0:P, cs:cs + cw], pm[ci][0:P, 0:cw],
                                            pb_bc[0:P, cs:cs + cw], ALU.add)
                nc.sync.dma_start(y_d[b * T + ts_: b * T + ts_ + P, :], ys[0:P, :])

        stage1(0)
        for b in range(1, nbatch):
            stage1(b)
            attn(b - 1)
        attn(nbatch - 1)
    return nc


def host_inputs(x_c, qkv_w, qkv_b, proj_w, proj_b):
    xT = np.ascontiguousarray(
        np.transpose(x_c.reshape(B, T, D), (0, 2, 1)).reshape(B * D, T))
    qkwT = np.ascontiguousarray(qkv_w[0:EQK].T)
    vwT = np.ascontiguousarray(qkv_w[EQK:2304].T)
    b_v = qkv_b[EQK:2304]
    pb_aug = proj_b + b_v @ proj_w.T
    pwT = np.concatenate([proj_w.T, pb_aug[None, :]], axis=0)
    qkb = np.ascontiguousarray(qkv_b[0:EQK].reshape(H, 128).T)
    return {
        "xT": xT.astype(np.float32),
        "qkwT": qkwT.astype(np.float32),
        "vwT": vwT.astype(np.float32),
        "pwT": pwT.astype(np.float32),
        "qkb": qkb.astype(np.float32),
    }


import sys as _sys
import numpy as _np

def _split_waits(nc, max_waits=1):
    import concourse.mybir as mybir
    nid = [0]
    for f in nc.m.functions:
        for bb in f.blocks:
            newlist = []; changed = False
            for ins in bb.instructions:
                si = getattr(ins, 'sync_info', None)
                if si is not None and si.on_wait is not None and len(si.on_wait) > max_waits:
                    waits = list(si.on_wait)
                    extra, keep = waits[:-max_waits], waits[-max_waits:]
                    for i in range(0, len(extra), max_waits):
                        nop = mybir.InstNoOp(name=f"I-ws-{nid[0]}", ins=[], outs=[],
                            engine=ins.engine,
                            sync_info=mybir.SyncInfo(on_wait=extra[i:i+max_waits], on_update=[]))
                        nid[0] += 1; newlist.append(nop); changed = True
                    si.on_wait = keep; ins.sync_info = si
                newlist.append(ins)
            if changed:
                bb.instructions = newlist


_NC_CACHE = {}

import os as _os
def _get_nc():
    if "nc" not in _NC_CACHE:
        import json as _json
        flags = _json.loads(_os.environ.get("V6_FLAGS", "{}"))
        nc = build(nbatch=B, **flags)
        _split_waits(nc)
        _NC_CACHE["nc"] = nc
    return _NC_CACHE["nc"]


def kernel(x, qkv_w, qkv_b, proj_w, proj_b):
    """Full inputs in ([32,577,768] etc.), full output out.

    Data-parallel over batch: 32 batches -> 8 NeuronCores x 4 each. Weights
    replicated (host-side transpose is layout prep only); all compute on
    device (Bass/Tile, f32r projections + bf16 attention, f32 accumulation,
    software-pipelined batch stages).
    """
    x = _np.asarray(x, dtype=_np.float32)
    qkv_w = _np.asarray(qkv_w, dtype=_np.float32)
    qkv_b = _np.asarray(qkv_b, dtype=_np.float32)
    proj_w = _np.asarray(proj_w, dtype=_np.float32)
    proj_b = _np.asarray(proj_b, dtype=_np.float32)
    from concourse.bass_utils import run_bass_kernel_spmd
    nc = _get_nc()
    in_maps = [host_inputs(x[c * B:(c + 1) * B], qkv_w, qkv_b, proj_w, proj_b)
               for c in range(8)]
    res = run_bass_kernel_spmd(nc, in_maps, list(range(8)))
    y = _np.concatenate([res.results[c]["y"].reshape(B, T, D) for c in range(8)], axis=0)
    return y.astype(_np.float32)


# revision 25
# speedup vs baseline: 1.0927x; 1.0927x over previous
"""v7: interleaved-phase schedule (stage1(b) | attn(b-1) | norm+MM4(b-2)),
host-transposed x, f32r direct-DMA projections, bf16 attention, paired MM2
row-groups, merged 2-bank PSUM for S and o, engine rebalance (gpsimd bias
and den-row copies, ACT ln/exp reciprocal), LDW-reuse loop orders."""
import numpy as np
import concourse.bass as bass
import concourse.mybir as mybir
import concourse.tile as tile

dt = mybir.dt
F32 = dt.float32
F32R = dt.float32r
BF16 = dt.bfloat16
AF = mybir.ActivationFunctionType
ALU = mybir.AluOpType

B = 4            # batches per core
T = 577
D = 768
H = 12
HD = 64
EQK = 1536
SCALE = HD ** -0.5
NTOK = B * T

TT = [(i * 128, min(128, T - i * 128)) for i in range((T + 127) // 128)]
TP = 578         # padded token count (free dim)
ICH = [(0, 320), (320, 258)]       # f32r moving chunks (>=256 each)
BCH = [(0, 512), (512, 66)]        # bf16 moving chunks (bank-aligned)
ECH = [(0, 384), (384, 384)]
DT = 6


def build(nbatch=B, gpsb=True, gpsdh=True, lnexp=True):
    nc = bass.Bass()
    xT_d = nc.dram_tensor("xT", [nbatch * D, T], F32R, kind="ExternalInput")
    qkwT_d = nc.dram_tensor("qkwT", [D, EQK], F32R, kind="ExternalInput")
    vwT_d = nc.dram_tensor("vwT", [D, D], F32R, kind="ExternalInput")
    pwT_d = nc.dram_tensor("pwT", [D + 1, D], F32, kind="ExternalInput")
    qkb_d = nc.dram_tensor("qkb", [128, H], F32, kind="ExternalInput")
    y_d = nc.dram_tensor("y", [NTOK, D], F32, kind="ExternalOutput")

    from contextlib import ExitStack
    with tile.TileContext(nc) as tc, ExitStack() as ctx:
        wpool = ctx.enter_context(tc.tile_pool(name="wpool", bufs=1))
        stg = ctx.enter_context(tc.tile_pool(name="stg", bufs=2))

        qkwT = []
        for dti in range(DT):
            w = wpool.tile([128, EQK], F32R, tag=f"qkwT{dti}", name=f"qkwT{dti}")
            nc.sync.dma_start(w[:], qkwT_d[dti * 128:(dti + 1) * 128, :])
            qkwT.append(w)
        vwT = []
        for dti in range(DT):
            w = wpool.tile([128, D], F32R, tag=f"vwT{dti}", name=f"vwT{dti}")
            nc.sync.dma_start(w[:], vwT_d[dti * 128:(dti + 1) * 128, :])
            vwT.append(w)
        pwT = []
        for dti in range(DT):
            w = wpool.tile([128, D], BF16, tag=f"pwT{dti}", name=f"pwT{dti}")
            s = stg.tile([128, D], F32, tag="wstage", name=f"sp{dti}")
            nc.sync.dma_start(s[:], pwT_d[dti * 128:(dti + 1) * 128, :])
            nc.vector.tensor_copy(w[:], s[:])
            pwT.append(w)
        pb_bc = wpool.tile([128, D], F32, tag="pb_bc")
        s = stg.tile([128, D], F32, tag="wstage", name="spb")
        nc.sync.dma_start(s[0:1, :], pwT_d[D:D + 1, :])
        pb_row = wpool.tile([1, D], F32, tag="pb_row")
        nc.vector.tensor_copy(pb_row[:], s[0:1, :])

        qkb_sb = wpool.tile([128, H], F32, tag="qkb")
        nc.sync.dma_start(qkb_sb[:], qkb_d[:])

        ones_col_f = wpool.tile([128, 1], F32, tag="ones_col_f")
        nc.gpsimd.memset(ones_col_f[:], 1.0)
        ones_col = wpool.tile([128, 1], BF16, tag="ones_col")
        nc.vector.tensor_copy(ones_col[:], ones_col_f[:])

        xT_p = ctx.enter_context(tc.tile_pool(name="xT", bufs=2))
        qkT_p = ctx.enter_context(tc.tile_pool(name="qkT", bufs=2))
        v_p = ctx.enter_context(tc.tile_pool(name="v", bufs=2))
        es_p = ctx.enter_context(tc.tile_pool(name="es", bufs=1))
        oT_p = ctx.enter_context(tc.tile_pool(name="oT", bufs=2))
        nrm_p = ctx.enter_context(tc.tile_pool(name="nrm", bufs=2))
        bc_p = ctx.enter_context(tc.tile_pool(name="bc", bufs=2))
        yout = ctx.enter_context(tc.tile_pool(name="yout", bufs=2))
        drp = ctx.enter_context(tc.tile_pool(name="dr", bufs=2, space="DRAM"))
        pbd = drp.tile([1, D], F32, tag="pbd")
        nc.sync.dma_start(pbd[:], pb_row[:])
        nc.sync.dma_start(pb_bc[:], pbd[0:1, :].to_broadcast((128, D)))

        ps_s = ctx.enter_context(tc.tile_pool(name="ps_s", bufs=2, space="PSUM"))
        ps_mm = ctx.enter_context(tc.tile_pool(name="ps_mm", bufs=2, space="PSUM"))
        ps_o = ctx.enter_context(tc.tile_pool(name="ps_o", bufs=1, space="PSUM"))

        qkTs, v_sbs, oTs, rdens = {}, {}, {}, {}

        def stage1_gen(b):
            xT = [xT_p.tile([128, TP], F32R, tag=f"xT{dti}", name=f"xT{dti}_{b}")
                  for dti in range(DT)]
            for dti in range(DT):
                nc.gpsimd.memset(xT[dti][:, T:TP].bitcast(F32), 0.0)
                nc.sync.dma_start(xT[dti][:, 0:T],
                                  xT_d[b * D + dti * 128: b * D + (dti + 1) * 128, :])
            qkT = [qkT_p.tile([128, TP], BF16, tag=f"qkT{et}", name=f"qkT{et}_{b}")
                   for et in range(12)]
            qkTs[b] = qkT
            v_sb = [v_p.tile([128, H * (HD + 1)], BF16, tag=f"v{ti}", name=f"v{ti}_{b}")
                    for ti in range(len(TT))]
            v_sbs[b] = v_sb
            yield
            # MM1a: one et per segment; dti-outer so each LDW serves 2 chunks
            for et in range(12):
                pm = [ps_mm.tile([128, 512], F32, tag="ps_mm", name=f"pma_{b}_{et}_{ci}")
                      for ci in range(2)]
                for dti in range(DT):
                    for ci, (cs, cw) in enumerate(ICH):
                        nc.tensor.matmul(pm[ci][:, 0:cw],
                                         qkwT[dti][:, et * 128:(et + 1) * 128],
                                         xT[dti][:, cs:cs + cw],
                                         start=(dti == 0), stop=(dti == DT - 1))
                for ci, (cs, cw) in enumerate(ICH):
                    if gpsb:
                        nc.scalar.activation(qkT[et][:, cs:cs + cw], pm[ci][:, 0:cw],
                                             AF.Identity, bias=qkb_sb[:, et:et + 1])
                    else:
                        nc.vector.tensor_scalar_add(qkT[et][:, cs:cs + cw],
                                                    pm[ci][:, 0:cw],
                                                    qkb_sb[:, et:et + 1])
                yield
            # MM1b: one ti per segment
            for ti, (ts_, P) in enumerate(TT):
                vv = v_sb[ti].rearrange("p (h c) -> p h c", c=HD + 1)
                nc.vector.tensor_copy(vv[0:P, :, HD:HD + 1],
                                      ones_col[0:P, :].to_broadcast((P, H, 1)))
                pm = [ps_mm.tile([128, 512], F32, tag="ps_mm", name=f"pmb_{b}_{ti}_{ci}")
                      for ci in range(2)]
                for dti in range(DT):
                    for ci, (cs, cw) in enumerate(ECH):
                        nc.tensor.matmul(pm[ci][0:P, 0:cw],
                                         xT[dti][:, ts_:ts_ + P],
                                         vwT[dti][:, cs:cs + cw],
                                         start=(dti == 0), stop=(dti == DT - 1))
                for ci, (cs, cw) in enumerate(ECH):
                    s3 = pm[ci][0:P, 0:cw].rearrange("p (h c) -> p h c", c=HD)
                    nc.vector.tensor_copy(vv[0:P, ci * 6:(ci + 1) * 6, 0:HD], s3)
                yield

        def attn_gen(b):
            qkT, v_sb = qkTs.pop(b), v_sbs.pop(b)
            oT = [oT_p.tile([128, TP], BF16, tag=f"oT{dti}", name=f"oT{dti}_{b}")
                  for dti in range(DT)]
            oTs[b] = oT
            rdr_den = drp.tile([H, TP], F32, tag="rdr_den", name=f"rdrden_{b}")
            rdens[b] = rdr_den
            for g in range(6):
                qt = qkT[g]
                kt = qkT[6 + g]
                es = {}
                for h01 in range(2):
                    h = 2 * g + h01
                    es[h01] = [es_p.tile([128, TP], BF16, tag=f"es{ji}_{h % 3}",
                                         name=f"es{ji}_{b}_{h}")
                               for ji in range(len(TT))]
                # paired MM2 (row groups 0-63 / 64-127 concurrent), merged exp
                for ji, (js, JP) in enumerate(TT):
                    pss = [ps_s.tile([128, TP], F32, tag="ps_s",
                                     name=f"pss_{b}_{g}_{ji}_{h01}")
                           for h01 in range(2)]
                    for (cs, cw) in BCH:
                        for h01 in range(2):
                            par = h01 * 64
                            nc.tensor.matmul(pss[h01][0:JP, cs:cs + cw],
                                             kt[par:par + 64, js:js + JP],
                                             qt[par:par + 64, cs:cs + cw],
                                             start=True, stop=True)
                    for h01 in range(2):
                        nc.scalar.activation(es[h01][ji][0:JP, 0:TP],
                                             pss[h01][0:JP, 0:TP],
                                             AF.Exp, scale=SCALE)
                # MM3 per head into one 2-bank PSUM tile; single evacs
                for h01 in range(2):
                    h = 2 * g + h01
                    par = h01 * 64
                    po = ps_o.tile([128, TP], F32, tag="ps_o", name=f"po_{b}_{h}")
                    for (cs, cw) in BCH:
                        for ji, (js, JP) in enumerate(TT):
                            nc.tensor.matmul(po[0:HD + 1, cs:cs + cw],
                                             v_sb[ji][0:JP, h * (HD + 1):(h + 1) * (HD + 1)],
                                             es[h01][ji][0:JP, cs:cs + cw],
                                             start=(ji == 0), stop=(ji == len(TT) - 1))
                    nc.vector.tensor_copy(oT[g][par:par + 64, 0:T], po[0:HD, 0:T])
                    dh = nrm_p.tile([1, TP], F32, tag="dh", name=f"dh_{b}_{h}")
                    nc.vector.tensor_copy(dh[0:1, 0:T], po[HD:HD + 1, 0:T])
                    nc.sync.dma_start(rdr_den[h:h + 1, 0:T], dh[0:1, 0:T])
                yield

        def nm4_gen(b):
            oT = oTs.pop(b)
            rdr_den = rdens.pop(b)
            # reciprocal of denominators: ACT ln/exp (same table set) or DVE
            den = nrm_p.tile([H, TP], F32, tag="den", name=f"den_{b}")
            nc.sync.dma_start(den[:, 0:T], rdr_den[:, 0:T])
            rec = nrm_p.tile([H, TP], F32, tag="rec", name=f"rec_{b}")
            if lnexp:
                lnd = nrm_p.tile([H, TP], F32, tag="lnd", name=f"lnd_{b}")
                nc.scalar.activation(lnd[:, 0:T], den[:, 0:T], AF.Ln)
                nc.scalar.activation(rec[:, 0:T], lnd[:, 0:T], AF.Exp, scale=-1.0)
            else:
                nc.vector.reciprocal(rec[:, 0:T], den[:, 0:T])
            rdr = drp.tile([H, TP], F32, tag="rdr", name=f"rdr_{b}")
            nc.sync.dma_start(rdr[:, 0:T], rec[:, 0:T])
            yield
            for g in range(6):
                bc = bc_p.tile([128, TP], F32, tag="bc", name=f"bc_{b}_{g}")
                eng = nc.sync if g % 2 == 0 else nc.scalar
                eng.dma_start(bc[0:64, 0:T],
                              rdr[2 * g:2 * g + 1, 0:T].to_broadcast((64, T)))
                eng.dma_start(bc[64:128, 0:T],
                              rdr[2 * g + 1:2 * g + 2, 0:T].to_broadcast((64, T)))
                nc.vector.tensor_tensor(oT[g][:, 0:T], oT[g][:, 0:T],
                                        bc[:, 0:T], ALU.mult)
                if g % 3 == 2:
                    yield
            # MM4: one ti per segment; dti-outer for LDW reuse
            for ti, (ts_, P) in enumerate(TT):
                ys = yout.tile([128, D], F32, tag="y_sb", name=f"ys_{b}_{ti}")
                pm = [ps_mm.tile([128, 512], F32, tag="ps_mm", name=f"pmc_{b}_{ti}_{ci}")
                      for ci in range(2)]
                for dti in range(DT):
                    for ci, (cs, cw) in enumerate(ECH):
                        nc.tensor.matmul(pm[ci][0:P, 0:cw],
                                         oT[dti][:, ts_:ts_ + P],
                                         pwT[dti][:, cs:cs + cw],
                                         start=(dti == 0), stop=(dti == DT - 1))
                for ci, (cs, cw) in enumerate(ECH):
                    nc.vector.tensor_tensor(ys[0:P, cs:cs + cw], pm[ci][0:P, 0:cw],
                                            pb_bc[0:P, cs:cs + cw], ALU.add)
                nc.sync.dma_start(y_d[b * T + ts_: b * T + ts_ + P, :], ys[0:P, :])
                yield

        def drain(gen, n=None):
            if gen is None:
                return
            k = 0
            for _ in gen:
                k += 1
                if n is not None and k >= n:
                    return

        # rounds: r runs stage1(r) | attn(r-1) | nm4(r-2), interleaved
        for r in range(nbatch + 2):
            s1 = stage1_gen(r) if r < nbatch else None
            at = attn_gen(r - 1) if 1 <= r <= nbatch else None
            nm = nm4_gen(r - 2) if 2 <= r <= nbatch + 1 else None
            if s1 is not None:
                drain(s1, 1)          # x DMAs + tile allocs first
            for g in range(6):
                drain(s1, 2)          # two MM1a et segments
                drain(nm, 2)          # norm / MM4 pieces of b-2
                drain(at, 1)          # attention head-pair group
                drain(s1, 1)          # MM1b ti segment (5 of 6 iters)
            drain(s1)
            drain(at)
            drain(nm)
    return nc


def host_inputs(x_c, qkv_w, qkv_b, proj_w, proj_b):
    xT = np.ascontiguousarray(
        np.transpose(x_c.reshape(B, T, D), (0, 2, 1)).reshape(B * D, T))
    qkwT = np.ascontiguousarray(qkv_w[0:EQK].T)
    vwT = np.ascontiguousarray(qkv_w[EQK:2304].T)
    b_v = qkv_b[EQK:2304]
    pb_aug = proj_b + b_v @ proj_w.T
    pwT = np.concatenate([proj_w.T, pb_aug[None, :]], axis=0)
    qkb = np.ascontiguousarray(qkv_b[0:EQK].reshape(H, 128).T)
    return {
        "xT": xT.astype(np.float32),
        "qkwT": qkwT.astype(np.float32),
        "vwT": vwT.astype(np.float32),
        "pwT": pwT.astype(np.float32),
        "qkb": qkb.astype(np.float32),
    }


import sys as _sys
import numpy as _np

def _split_waits(nc, max_waits=1):
    import concourse.mybir as mybir
    nid = [0]
    for f in nc.m.functions:
        for bb in f.blocks:
            newlist = []; changed = False
            for ins in bb.instructions:
                si = getattr(ins, 'sync_info', None)
                if si is not None and si.on_wait is not None and len(si.on_wait) > max_waits:
                    waits = list(si.on_wait)
                    extra, keep = waits[:-max_waits], waits[-max_waits:]
                    for i in range(0, len(extra), max_waits):
                        nop = mybir.InstNoOp(name=f"I-ws-{nid[0]}", ins=[], outs=[],
                            engine=ins.engine,
                            sync_info=mybir.SyncInfo(on_wait=extra[i:i+max_waits], on_update=[]))
                        nid[0] += 1; newlist.append(nop); changed = True
                    si.on_wait = keep; ins.sync_info = si
                newlist.append(ins)
            if changed:
                bb.instructions = newlist


_NC_CACHE = {}

import os as _os
def _get_nc():
    if "nc" not in _NC_CACHE:
        import json as _json
        flags = _json.loads(_os.environ.get("V7_FLAGS", "{}"))
        nc = build(nbatch=B, **flags)
        _split_waits(nc)
        _NC_CACHE["nc"] = nc
    return _NC_CACHE["nc"]


def kernel(x, qkv_w, qkv_b, proj_w, proj_b):
    """Full inputs in ([32,577,768] etc.), full output out.

    Data-parallel over batch: 32 batches -> 8 NeuronCores x 4 each. Weights
    replicated (host-side transpose is layout prep only); all compute on
    device (Bass/Tile, f32r projections + bf16 attention, f32 accumulation,
    interleaved software-pipelined batch stages).
    """
    x = _np.asarray(x, dtype=_np.float32)
    qkv_w = _np.asarray(qkv_w, dtype=_np.float32)
    qkv_b = _np.asarray(qkv_b, dtype=_np.float32)
    proj_w = _np.asarray(proj_w, dtype=_np.float32)
    proj_b = _np.asarray(proj_b, dtype=_np.float32)
    from concourse.bass_utils import run_bass_kernel_spmd
    nc = _get_nc()
    in_maps = [host_inputs(x[c * B:(c + 1) * B], qkv_w, qkv_b, proj_w, proj_b)
               for c in range(8)]
    res = run_bass_kernel_spmd(nc, in_maps, list(range(8)))
    y = _np.concatenate([res.results[c]["y"].reshape(B, T, D) for c in range(8)], axis=0)
    return y.astype(_np.float32)


# revision 26
# speedup vs baseline: 1.1326x; 1.0364x over previous
"""v7: interleaved-phase schedule (stage1(b) | attn(b-1) | norm+MM4(b-2)),
host-transposed x, f32r direct-DMA projections, bf16 attention, paired MM2
row-groups, merged 2-bank PSUM for S and o, engine rebalance (gpsimd bias
and den-row copies, ACT ln/exp reciprocal), LDW-reuse loop orders."""
import numpy as np
import concourse.bass as bass
import concourse.mybir as mybir
import concourse.tile as tile

dt = mybir.dt
F32 = dt.float32
F32R = dt.float32r
BF16 = dt.bfloat16
AF = mybir.ActivationFunctionType
ALU = mybir.AluOpType

B = 4            # batches per core
T = 577
D = 768
H = 12
HD = 64
EQK = 1536
SCALE = HD ** -0.5
NTOK = B * T

TT = [(i * 128, min(128, T - i * 128)) for i in range((T + 127) // 128)]
TP = 578         # padded token count (free dim)
ICH = [(0, 320), (320, 258)]       # f32r moving chunks (>=256 each)
BCH = [(0, 512), (512, 66)]        # bf16 moving chunks (bank-aligned)
ECH = [(0, 384), (384, 384)]
DT = 6


def build(nbatch=B, gpsb=False, gpsdh=True, lnexp=True):
    nc = bass.Bass()
    xT_d = nc.dram_tensor("xT", [nbatch * D, T], F32R, kind="ExternalInput")
    qkwT_d = nc.dram_tensor("qkwT", [D, EQK], F32R, kind="ExternalInput")
    vwT_d = nc.dram_tensor("vwT", [D, D], F32R, kind="ExternalInput")
    pwT_d = nc.dram_tensor("pwT", [D + 1, D], F32, kind="ExternalInput")
    qkb_d = nc.dram_tensor("qkb", [128, H], F32, kind="ExternalInput")
    y_d = nc.dram_tensor("y", [NTOK, D], F32, kind="ExternalOutput")

    from contextlib import ExitStack
    with tile.TileContext(nc) as tc, ExitStack() as ctx:
        wpool = ctx.enter_context(tc.tile_pool(name="wpool", bufs=1))
        stg = ctx.enter_context(tc.tile_pool(name="stg", bufs=2))

        qkwT = []
        for dti in range(DT):
            w = wpool.tile([128, EQK], F32R, tag=f"qkwT{dti}", name=f"qkwT{dti}")
            nc.sync.dma_start(w[:, 0:256], qkwT_d[dti * 128:(dti + 1) * 128, 0:256])
            qkwT.append(w)
        for dti in range(DT):
            nc.sync.dma_start(qkwT[dti][:, 256:EQK],
                              qkwT_d[dti * 128:(dti + 1) * 128, 256:EQK])
        vwT = []
        for dti in range(DT):
            w = wpool.tile([128, D], F32R, tag=f"vwT{dti}", name=f"vwT{dti}")
            nc.sync.dma_start(w[:], vwT_d[dti * 128:(dti + 1) * 128, :])
            vwT.append(w)
        pwT = []
        for dti in range(DT):
            w = wpool.tile([128, D], BF16, tag=f"pwT{dti}", name=f"pwT{dti}")
            s = stg.tile([128, D], F32, tag="wstage", name=f"sp{dti}")
            nc.sync.dma_start(s[:], pwT_d[dti * 128:(dti + 1) * 128, :])
            nc.vector.tensor_copy(w[:], s[:])
            pwT.append(w)
        pb_bc = wpool.tile([128, D], F32, tag="pb_bc")
        s = stg.tile([128, D], F32, tag="wstage", name="spb")
        nc.sync.dma_start(s[0:1, :], pwT_d[D:D + 1, :])
        pb_row = wpool.tile([1, D], F32, tag="pb_row")
        nc.vector.tensor_copy(pb_row[:], s[0:1, :])

        qkb_sb = wpool.tile([128, H], F32, tag="qkb")
        nc.sync.dma_start(qkb_sb[:], qkb_d[:])

        ones_col_f = wpool.tile([128, 1], F32, tag="ones_col_f")
        nc.gpsimd.memset(ones_col_f[:], 1.0)
        ones_col = wpool.tile([128, 1], BF16, tag="ones_col")
        nc.vector.tensor_copy(ones_col[:], ones_col_f[:])

        xT_p = ctx.enter_context(tc.tile_pool(name="xT", bufs=2))
        qkT_p = ctx.enter_context(tc.tile_pool(name="qkT", bufs=2))
        v_p = ctx.enter_context(tc.tile_pool(name="v", bufs=2))
        es_p = ctx.enter_context(tc.tile_pool(name="es", bufs=1))
        oT_p = ctx.enter_context(tc.tile_pool(name="oT", bufs=2))
        nrm_p = ctx.enter_context(tc.tile_pool(name="nrm", bufs=2))
        bc_p = ctx.enter_context(tc.tile_pool(name="bc", bufs=2))
        yout = ctx.enter_context(tc.tile_pool(name="yout", bufs=2))
        drp = ctx.enter_context(tc.tile_pool(name="dr", bufs=2, space="DRAM"))
        pbd = drp.tile([1, D], F32, tag="pbd")
        nc.sync.dma_start(pbd[:], pb_row[:])
        nc.sync.dma_start(pb_bc[:], pbd[0:1, :].to_broadcast((128, D)))

        ps_s = ctx.enter_context(tc.tile_pool(name="ps_s", bufs=2, space="PSUM"))
        ps_mm = ctx.enter_context(tc.tile_pool(name="ps_mm", bufs=2, space="PSUM"))
        ps_o = ctx.enter_context(tc.tile_pool(name="ps_o", bufs=1, space="PSUM"))

        qkTs, v_sbs, oTs, rdens = {}, {}, {}, {}

        def stage1_gen(b):
            xT = [xT_p.tile([128, TP], F32R, tag=f"xT{dti}", name=f"xT{dti}_{b}")
                  for dti in range(DT)]
            for dti in range(DT):
                nc.gpsimd.memset(xT[dti][:, T:TP].bitcast(F32), 0.0)
                nc.sync.dma_start(xT[dti][:, 0:T],
                                  xT_d[b * D + dti * 128: b * D + (dti + 1) * 128, :])
            qkT = [qkT_p.tile([128, TP], BF16, tag=f"qkT{et}", name=f"qkT{et}_{b}")
                   for et in range(12)]
            qkTs[b] = qkT
            v_sb = [v_p.tile([128, H * (HD + 1)], BF16, tag=f"v{ti}", name=f"v{ti}_{b}")
                    for ti in range(len(TT))]
            v_sbs[b] = v_sb
            yield
            # MM1a: one et per segment; dti-outer so each LDW serves 2 chunks
            for et in range(12):
                pm = [ps_mm.tile([128, 512], F32, tag="ps_mm", name=f"pma_{b}_{et}_{ci}")
                      for ci in range(2)]
                for dti in range(DT):
                    for ci, (cs, cw) in enumerate(ICH):
                        nc.tensor.matmul(pm[ci][:, 0:cw],
                                         qkwT[dti][:, et * 128:(et + 1) * 128],
                                         xT[dti][:, cs:cs + cw],
                                         start=(dti == 0), stop=(dti == DT - 1))
                for ci, (cs, cw) in enumerate(ICH):
                    if gpsb:
                        nc.scalar.activation(qkT[et][:, cs:cs + cw], pm[ci][:, 0:cw],
                                             AF.Identity, bias=qkb_sb[:, et:et + 1])
                    else:
                        nc.vector.tensor_scalar_add(qkT[et][:, cs:cs + cw],
                                                    pm[ci][:, 0:cw],
                                                    qkb_sb[:, et:et + 1])
                yield
            # MM1b: one ti per segment
            for ti, (ts_, P) in enumerate(TT):
                vv = v_sb[ti].rearrange("p (h c) -> p h c", c=HD + 1)
                nc.vector.tensor_copy(vv[0:P, :, HD:HD + 1],
                                      ones_col[0:P, :].to_broadcast((P, H, 1)))
                pm = [ps_mm.tile([128, 512], F32, tag="ps_mm", name=f"pmb_{b}_{ti}_{ci}")
                      for ci in range(2)]
                for dti in range(DT):
                    for ci, (cs, cw) in enumerate(ECH):
                        nc.tensor.matmul(pm[ci][0:P, 0:cw],
                                         xT[dti][:, ts_:ts_ + P],
                                         vwT[dti][:, cs:cs + cw],
                                         start=(dti == 0), stop=(dti == DT - 1))
                for ci, (cs, cw) in enumerate(ECH):
                    s3 = pm[ci][0:P, 0:cw].rearrange("p (h c) -> p h c", c=HD)
                    nc.vector.tensor_copy(vv[0:P, ci * 6:(ci + 1) * 6, 0:HD], s3)
                yield

        def attn_gen(b):
            qkT, v_sb = qkTs.pop(b), v_sbs.pop(b)
            oT = [oT_p.tile([128, TP], BF16, tag=f"oT{dti}", name=f"oT{dti}_{b}")
                  for dti in range(DT)]
            oTs[b] = oT
            rdr_den = drp.tile([H, TP], F32, tag="rdr_den", name=f"rdrden_{b}")
            rdens[b] = rdr_den
            for g in range(6):
                qt = qkT[g]
                kt = qkT[6 + g]
                es = {}
                for h01 in range(2):
                    h = 2 * g + h01
                    es[h01] = [es_p.tile([128, TP], BF16, tag=f"es{ji}_{h % 3}",
                                         name=f"es{ji}_{b}_{h}")
                               for ji in range(len(TT))]
                # paired MM2 (row groups 0-63 / 64-127 concurrent), merged exp
                for ji, (js, JP) in enumerate(TT):
                    pss = [ps_s.tile([128, TP], F32, tag="ps_s",
                                     name=f"pss_{b}_{g}_{ji}_{h01}")
                           for h01 in range(2)]
                    for (cs, cw) in BCH:
                        for h01 in range(2):
                            par = h01 * 64
                            nc.tensor.matmul(pss[h01][0:JP, cs:cs + cw],
                                             kt[par:par + 64, js:js + JP],
                                             qt[par:par + 64, cs:cs + cw],
                                             start=True, stop=True)
                    for h01 in range(2):
                        nc.scalar.activation(es[h01][ji][0:JP, 0:TP],
                                             pss[h01][0:JP, 0:TP],
                                             AF.Exp, scale=SCALE)
                yield
                # MM3 per head into one 2-bank PSUM tile; single evacs
                for h01 in range(2):
                    h = 2 * g + h01
                    par = h01 * 64
                    po = ps_o.tile([128, TP], F32, tag="ps_o", name=f"po_{b}_{h}")
                    for (cs, cw) in BCH:
                        for ji, (js, JP) in enumerate(TT):
                            nc.tensor.matmul(po[0:HD + 1, cs:cs + cw],
                                             v_sb[ji][0:JP, h * (HD + 1):(h + 1) * (HD + 1)],
                                             es[h01][ji][0:JP, cs:cs + cw],
                                             start=(ji == 0), stop=(ji == len(TT) - 1))
                    nc.vector.tensor_copy(oT[g][par:par + 64, 0:T], po[0:HD, 0:T])
                    dh = nrm_p.tile([1, TP], F32, tag="dh", name=f"dh_{b}_{h}")
                    nc.vector.tensor_copy(dh[0:1, 0:T], po[HD:HD + 1, 0:T])
                    nc.sync.dma_start(rdr_den[h:h + 1, 0:T], dh[0:1, 0:T])
                yield

        def nm4_gen(b):
            oT = oTs.pop(b)
            rdr_den = rdens.pop(b)
            # reciprocal of denominators: ACT ln/exp (same table set) or DVE
            den = nrm_p.tile([H, TP], F32, tag="den", name=f"den_{b}")
            nc.sync.dma_start(den[:, 0:T], rdr_den[:, 0:T])
            rec = nrm_p.tile([H, TP], F32, tag="rec", name=f"rec_{b}")
            if lnexp:
                lnd = nrm_p.tile([H, TP], F32, tag="lnd", name=f"lnd_{b}")
                nc.scalar.activation(lnd[:, 0:T], den[:, 0:T], AF.Ln)
                nc.scalar.activation(rec[:, 0:T], lnd[:, 0:T], AF.Exp, scale=-1.0)
            else:
                nc.vector.reciprocal(rec[:, 0:T], den[:, 0:T])
            rdr = drp.tile([H, TP], F32, tag="rdr", name=f"rdr_{b}")
            nc.sync.dma_start(rdr[:, 0:T], rec[:, 0:T])
            yield
            for g in range(6):
                bc = bc_p.tile([128, TP], F32, tag="bc", name=f"bc_{b}_{g}")
                eng = nc.sync if g % 2 == 0 else nc.scalar
                eng.dma_start(bc[0:64, 0:T],
                              rdr[2 * g:2 * g + 1, 0:T].to_broadcast((64, T)))
                eng.dma_start(bc[64:128, 0:T],
                              rdr[2 * g + 1:2 * g + 2, 0:T].to_broadcast((64, T)))
                nc.vector.tensor_tensor(oT[g][:, 0:T], oT[g][:, 0:T],
                                        bc[:, 0:T], ALU.mult)
                if g % 3 == 2:
                    yield
            # MM4: one ti per segment; dti-outer for LDW reuse
            for ti, (ts_, P) in enumerate(TT):
                ys = yout.tile([128, D], F32, tag="y_sb", name=f"ys_{b}_{ti}")
                pm = [ps_mm.tile([128, 512], F32, tag="ps_mm", name=f"pmc_{b}_{ti}_{ci}")
                      for ci in range(2)]
                for dti in range(DT):
                    for ci, (cs, cw) in enumerate(ECH):
                        nc.tensor.matmul(pm[ci][0:P, 0:cw],
                                         oT[dti][:, ts_:ts_ + P],
                                         pwT[dti][:, cs:cs + cw],
                                         start=(dti == 0), stop=(dti == DT - 1))
                for ci, (cs, cw) in enumerate(ECH):
                    nc.vector.tensor_tensor(ys[0:P, cs:cs + cw], pm[ci][0:P, 0:cw],
                                            pb_bc[0:P, cs:cs + cw], ALU.add)
                nc.sync.dma_start(y_d[b * T + ts_: b * T + ts_ + P, :], ys[0:P, :])
                yield

        def drain(gen, n=None):
            if gen is None:
                return
            k = 0
            for _ in gen:
                k += 1
                if n is not None and k >= n:
                    return

        # rounds: r runs stage1(r) | attn(r-1) | nm4(r-2), interleaved
        for r in range(nbatch + 2):
            s1 = stage1_gen(r) if r < nbatch else None
            at = attn_gen(r - 1) if 1 <= r <= nbatch else None
            nm = nm4_gen(r - 2) if 2 <= r <= nbatch + 1 else None
            if s1 is not None:
                drain(s1, 1)          # x DMAs + tile allocs first
            for g in range(6):
                drain(s1, 1)          # MM1a et segment
                drain(at, 1)          # attention MM2+exp of group g
                drain(s1, 1)          # MM1a et segment
                drain(nm, 2)          # norm / MM4 pieces of b-2
                drain(at, 1)          # attention MM3 pair of group g
                drain(s1, 1)          # MM1b ti segment (5 of 6 iters)
            drain(s1)
            drain(at)
            drain(nm)
    return nc


def host_inputs(x_c, qkv_w, qkv_b, proj_w, proj_b):
    xT = np.ascontiguousarray(
        np.transpose(x_c.reshape(B, T, D), (0, 2, 1)).reshape(B * D, T))
    qkwT = np.ascontiguousarray(qkv_w[0:EQK].T)
    vwT = np.ascontiguousarray(qkv_w[EQK:2304].T)
    b_v = qkv_b[EQK:2304]
    pb_aug = proj_b + b_v @ proj_w.T
    pwT = np.concatenate([proj_w.T, pb_aug[None, :]], axis=0)
    qkb = np.ascontiguousarray(qkv_b[0:EQK].reshape(H, 128).T)
    return {
        "xT": xT.astype(np.float32),
        "qkwT": qkwT.astype(np.float32),
        "vwT": vwT.astype(np.float32),
        "pwT": pwT.astype(np.float32),
        "qkb": qkb.astype(np.float32),
    }


import sys as _sys
import numpy as _np

def _split_waits(nc, max_waits=1):
    import concourse.mybir as mybir
    nid = [0]
    for f in nc.m.functions:
        for bb in f.blocks:
            newlist = []; changed = False
            for ins in bb.instructions:
                si = getattr(ins, 'sync_info', None)
                if si is not None and si.on_wait is not None and len(si.on_wait) > max_waits:
                    waits = list(si.on_wait)
                    extra, keep = waits[:-max_waits], waits[-max_waits:]
                    for i in range(0, len(extra), max_waits):
                        nop = mybir.InstNoOp(name=f"I-ws-{nid[0]}", ins=[], outs=[],
                            engine=ins.engine,
                            sync_info=mybir.SyncInfo(on_wait=extra[i:i+max_waits], on_update=[]))
                        nid[0] += 1; newlist.append(nop); changed = True
                    si.on_wait = keep; ins.sync_info = si
                newlist.append(ins)
            if changed:
                bb.instructions = newlist


_NC_CACHE = {}

import os as _os
def _get_nc():
    if "nc" not in _NC_CACHE:
        import json as _json
        flags = _json.loads(_os.environ.get("V7_FLAGS", "{}"))
        nc = build(nbatch=B, **flags)
        _split_waits(nc)
        _NC_CACHE["nc"] = nc
    return _NC_CACHE["nc"]


def kernel(x, qkv_w, qkv_b, proj_w, proj_b):
    """Full inputs in ([32,577,768] etc.), full output out.

    Data-parallel over batch: 32 batches -> 8 NeuronCores x 4 each. Weights
    replicated (host-side transpose is layout prep only); all compute on
    device (Bass/Tile, f32r projections + bf16 attention, f32 accumulation,
    interleaved software-pipelined batch stages).
    """
    x = _np.asarray(x, dtype=_np.float32)
    qkv_w = _np.asarray(qkv_w, dtype=_np.float32)
    qkv_b = _np.asarray(qkv_b, dtype=_np.float32)
    proj_w = _np.asarray(proj_w, dtype=_np.float32)
    proj_b = _np.asarray(proj_b, dtype=_np.float32)
    from concourse.bass_utils import run_bass_kernel_spmd
    nc = _get_nc()
    in_maps = [host_inputs(x[c * B:(c + 1) * B], qkv_w, qkv_b, proj_w, proj_b)
               for c in range(8)]
    res = run_bass_kernel_spmd(nc, in_maps, list(range(8)))
    y = _np.concatenate([res.results[c]["y"].reshape(B, T, D) for c in range(8)], axis=0)
    return y.astype(_np.float32)
